# revision 1
# baseline (speedup 1.0000x reference)
"""Trainium2 Bass kernel for nn_ATDecoder (GNN edge decoder).

Math:  out[e] = sigmoid(W2 @ LeakyReLU(W1l@z[src[e]] + b1l + W1r@z[dst[e]] + b1r) + b2)

Strategy (8 cores, SPMD):
  Phase 1 (replicated on every core): per-node transform.
      hl[n] = z[n] @ W1l.T + b1l,  hr[n] = z[n] @ W1r.T + b1r
    stored as fp16 tables in DRAM, in a permuted row order that makes the
    stores perfectly linear.  Computed from a host-transposed zT (bf16):
    lhsT = zT block [128ch, 128n], rhs = W1lT/W1rT [128ch, 64]; biases are
    folded in via a rank-1 matmul (lhsT = ones[1,128], rhs = bias row).
  Phase 2 (edges sharded, 75000/core): gather node vectors with the custom
    SWDGE dma_gather (InstDMAGatherAnt).  It needs int16 indices and >=256B
    rows, so tables are viewed as PAIR rows [50176, 128] (two nodes per
    256B row) split into 2 banks of <=32768 rows, and the host groups each
    core's edges into 16 classes by (src bank, src parity, dst bank, dst
    parity).  Per class: two gathers (classes are uniform so the needed
    64-wide half of each 128-wide pair row is a fixed strided slice),
    fused LeakyReLU max(0.01t, t) on DVE, multiply by broadcast W2,
    binary-tree reduce over the 64 hidden channels, sigmoid(+b2) on ACT.
    Host un-permutes the per-core [128, NCOL_OUT] outputs to edge order.
"""

import numpy as np

import concourse.bass as bass
import concourse.bacc as bacc
import concourse.mybir as mybir
from concourse.bass_utils import run_bass_kernel_spmd
from concourse.tile import TileContext

# ---------------------------------------------------------------- constants
P = 128
IN_CH = 128
HID = 64
N_NODES = 100000
N_EDGES = 600000
N_CORES = 8
NEG_SLOPE = 0.01

ROUND = 1024                         # nodes per phase-1 round
NR = (N_NODES + ROUND - 1) // ROUND  # 98 rounds
NPAD = NR * ROUND                    # 100352 padded nodes
NPAIR = NPAD // 2                    # 50176 pair rows
BANK = NPAIR // 2                    # 25088 pair rows per gather bank (balanced)
E_CORE = N_EDGES // N_CORES          # 75000 edges per core

NCLS = 16
C_SEG = 40                           # segments (of 128 slots) per class
CAP = C_SEG * P                      # 5376 slots per class
SW = CAP // 16                       # wrapped-idx cols per class (336)
NCOL_OUT = NCLS * C_SEG              # 672 output cols

BF16 = mybir.dt.bfloat16
FP16 = mybir.dt.float16
FP32 = mybir.dt.float32
I16 = mybir.dt.int16

_CACHE = {}


def build_nc():
    """Build the single-core Bass program (same program runs on all 8 cores)."""
    nc = bacc.Bacc()

    zT = nc.declare_dram_parameter("zT", [P, NPAD], BF16, isOutput=False)
    w1lT = nc.declare_dram_parameter("w1lT", [IN_CH, HID], BF16, isOutput=False)
    w1rT = nc.declare_dram_parameter("w1rT", [IN_CH, HID], BF16, isOutput=False)
    ones_r = nc.declare_dram_parameter("ones_r", [1, P], BF16, isOutput=False)
    bias_l = nc.declare_dram_parameter("bias_l", [1, 512], BF16, isOutput=False)
    bias_r = nc.declare_dram_parameter("bias_r", [1, 512], BF16, isOutput=False)
    w2full = nc.declare_dram_parameter("w2full", [P, C_SEG * HID], FP16, isOutput=False)
    b2col = nc.declare_dram_parameter("b2col", [P, 1], FP32, isOutput=False)
    src_w = nc.declare_dram_parameter("src_w", [P, NCLS * SW], I16, isOutput=False)
    dst_w = nc.declare_dram_parameter("dst_w", [P, NCLS * SW], I16, isOutput=False)
    out_d = nc.declare_dram_parameter("out", [P, NCOL_OUT], FP32, isOutput=True)

    hl_tab = nc.dram_tensor("hl_tab", [NPAIR, 2 * HID], FP16)
    hr_tab = nc.dram_tensor("hr_tab", [NPAIR, 2 * HID], FP16)

    with TileContext(nc) as tc:
        with (
            tc.tile_pool(name="const", bufs=1) as cpool,
            tc.tile_pool(name="zt", bufs=3) as ztpool,
            tc.tile_pool(name="ps", bufs=4, space="PSUM") as pspool,
            tc.tile_pool(name="stage", bufs=4) as stpool,
            tc.tile_pool(name="gath", bufs=2) as gpool,
            tc.tile_pool(name="prod", bufs=2) as ppool,
        ):
            # ---- constants into SBUF (edge indices prefetch too) ----
            w1lT_t = cpool.tile([IN_CH, HID], BF16, tag="w1l")
            nc.sync.dma_start(out=w1lT_t[:], in_=w1lT[:])
            w1rT_t = cpool.tile([IN_CH, HID], BF16, tag="w1r")
            nc.sync.dma_start(out=w1rT_t[:], in_=w1rT[:])
            ones_t = cpool.tile([1, P], BF16, tag="ones")
            nc.sync.dma_start(out=ones_t[:], in_=ones_r[:])
            bias_l_t = cpool.tile([1, 512], BF16, tag="bl")
            nc.sync.dma_start(out=bias_l_t[:], in_=bias_l[:])
            bias_r_t = cpool.tile([1, 512], BF16, tag="br")
            nc.sync.dma_start(out=bias_r_t[:], in_=bias_r[:])
            w2_t = cpool.tile([P, C_SEG * HID], FP16, tag="w2")
            nc.sync.dma_start(out=w2_t[:], in_=w2full[:])
            b2_t = cpool.tile([P, 1], FP32, tag="b2")
            nc.sync.dma_start(out=b2_t[:], in_=b2col[:])
            src_t = cpool.tile([P, NCLS * SW], I16, tag="srci")
            nc.sync.dma_start(out=src_t[:], in_=src_w[:])
            dst_t = cpool.tile([P, NCLS * SW], I16, tag="dsti")
            nc.sync.dma_start(out=dst_t[:], in_=dst_w[:])
            out_sb = cpool.tile([P, NCOL_OUT], FP32, tag="outsb")

            # ---------------- phase 1: node transform ----------------
            for r in range(NR):
                zt = ztpool.tile([P, ROUND], BF16, tag="zt")
                nc.sync.dma_start(out=zt[:], in_=zT[:, r * ROUND:(r + 1) * ROUND])

                psL = pspool.tile([P, 512], FP32, tag="psL")
                psR = pspool.tile([P, 512], FP32, tag="psR")
                # rank-1 bias fill: out[m, n] = ones.T @ bias_row = bias[n]
                nc.tensor.matmul(out=psL[:], lhsT=ones_t[:], rhs=bias_l_t[:],
                                 start=True, stop=False)
                nc.tensor.matmul(out=psR[:], lhsT=ones_t[:], rhs=bias_r_t[:],
                                 start=True, stop=False)
                for j in range(8):
                    ltj = zt[:, j * P:(j + 1) * P]
                    last = j == 7
                    nc.tensor.matmul(out=psL[:, j * HID:(j + 1) * HID],
                                     lhsT=ltj, rhs=w1lT_t[:],
                                     start=False, stop=last)
                    nc.tensor.matmul(out=psR[:, j * HID:(j + 1) * HID],
                                     lhsT=ltj, rhs=w1rT_t[:],
                                     start=False, stop=last)

                stL = stpool.tile([P, 512], FP16, tag="stL")
                stR = stpool.tile([P, 512], FP16, tag="stR")
                nc.scalar.activation(out=stL[:], in_=psL[:],
                                     func=mybir.ActivationFunctionType.Copy)
                nc.scalar.activation(out=stR[:], in_=psR[:],
                                     func=mybir.ActivationFunctionType.Copy)

                # permuted table rows: node n=r*1024+j*128+p -> 64-wide row
                # r*1024+p*8+j, i.e. a plain contiguous [128, 512] block.
                dl = hl_tab[r * 512:(r + 1) * 512, :].rearrange(
                    "(p j) d -> p (j d)", p=P)
                dr = hr_tab[r * 512:(r + 1) * 512, :].rearrange(
                    "(p j) d -> p (j d)", p=P)
                nc.sync.dma_start(out=dl, in_=stL[:])
                nc.sync.dma_start(out=dr, in_=stR[:])

            # ---------------- phase boundary ----------------
            tc.strict_bb_all_engine_barrier()

            # ---------------- phase 2: per-edge ----------------
            for c in range(NCLS):
                sb, sp, db, dp = (c >> 3) & 1, (c >> 2) & 1, (c >> 1) & 1, c & 1
                hl_bank = hl_tab[sb * BANK:(sb + 1) * BANK, :]
                hr_bank = hr_tab[db * BANK:(db + 1) * BANK, :]

                hlg = gpool.tile([P, C_SEG * 2 * HID], FP16, tag="hlg")
                hrg = gpool.tile([P, C_SEG * 2 * HID], FP16, tag="hrg")
                nc.gpsimd.dma_gather(
                    out_ap=hlg[:].rearrange("p (k d) -> p k d", d=2 * HID),
                    in_ap=hl_bank,
                    idxs_ap=src_t[:, c * SW:(c + 1) * SW],
                    num_idxs=CAP, num_idxs_reg=CAP, elem_size=2 * HID,
                    single_packet=False)
                nc.gpsimd.dma_gather(
                    out_ap=hrg[:].rearrange("p (k d) -> p k d", d=2 * HID),
                    in_ap=hr_bank,
                    idxs_ap=dst_t[:, c * SW:(c + 1) * SW],
                    num_idxs=CAP, num_idxs_reg=CAP, elem_size=2 * HID,
                    single_packet=False)

                hlg3 = hlg[:].rearrange("p (k d) -> p k d", d=2 * HID)
                hrg3 = hrg[:].rearrange("p (k d) -> p k d", d=2 * HID)
                t = ppool.tile([P, C_SEG * HID], FP16, tag="t")
                t3 = t[:].rearrange("p (k d) -> p k d", d=HID)
                # t = hl-half + hr-half  (halves chosen by class parity)
                nc.vector.tensor_tensor(
                    out=t3, in0=hlg3[:, :, sp * HID:(sp + 1) * HID],
                    in1=hrg3[:, :, dp * HID:(dp + 1) * HID],
                    op=mybir.AluOpType.add)
                # LeakyReLU in one fused op: max(0.01*t, t)
                nc.vector.scalar_tensor_tensor(
                    out=t[:], in0=t[:], scalar=NEG_SLOPE, in1=t[:],
                    op0=mybir.AluOpType.mult, op1=mybir.AluOpType.max)
                # multiply by broadcast W2
                nc.vector.tensor_tensor(
                    out=t[:], in0=t[:], in1=w2_t[:], op=mybir.AluOpType.mult)
                # tree-reduce the 64 hidden channels
                w = HID // 2
                while w >= 1:
                    nc.vector.tensor_tensor(
                        out=t3[:, :, 0:w], in0=t3[:, :, 0:w],
                        in1=t3[:, :, w:2 * w], op=mybir.AluOpType.add)
                    w //= 2
                # sigmoid(dot + b2)
                dots = t3[:, :, 0:1].rearrange("p k d -> p (k d)")
                nc.scalar.activation(
                    out=out_sb[:, c * C_SEG:(c + 1) * C_SEG], in_=dots,
                    func=mybir.ActivationFunctionType.Sigmoid,
                    bias=b2_t[:, 0:1], scale=1.0)

            nc.sync.dma_start(out=out_d[:], in_=out_sb[:])

    nc.finalize()
    return nc


# ------------------------------------------------------------- host helpers
def _node_perm():
    """pi[n] = permuted 64-wide table row for node n (vectorized)."""
    n = np.arange(NPAD, dtype=np.int64)
    base = (n // ROUND) * ROUND
    rr = n % ROUND
    j = rr // P
    p = rr % P
    return (base + p * 8 + j).astype(np.int64)


def _pack_core(srct, dstt):
    """Group one core's edges (pi-space table rows) into 16 classes.

    Returns (src_w [P, NCLS*SW] i16, dst_w [P, NCLS*SW] i16,
             part [E_CORE], col [E_CORE])  -- per-edge output coordinates.
    """
    spair, spar = srct >> 1, (srct & 1).astype(np.int64)
    dpair, dpar = dstt >> 1, (dstt & 1).astype(np.int64)
    sb = (spair >= BANK).astype(np.int64)
    db = (dpair >= BANK).astype(np.int64)
    cls = sb * 8 + spar * 4 + db * 2 + dpar

    order = np.argsort(cls, kind="stable")
    cnt = np.bincount(cls, minlength=NCLS)
    if cnt.max() > CAP:
        raise RuntimeError(f"class overflow: {cnt.max()} > {CAP}")
    start = np.cumsum(cnt) - cnt
    local_sorted = np.arange(srct.size) - start[cls[order]]
    # per-edge (original order) class + local slot
    local = np.empty(srct.size, dtype=np.int64)
    local[order] = local_sorted
    part = local % P
    col = cls * C_SEG + local // P

    def wrapped(vals_pair, bank_flags):
        out = np.zeros((NCLS, CAP), dtype=np.int16)
        v = (vals_pair - bank_flags * BANK).astype(np.int16)
        vs = v[order]
        for c in range(NCLS):
            out[c, :cnt[c]] = vs[start[c]:start[c] + cnt[c]]
        # wrapped layout: idx i at [i % 16, i // 16]; replicate to 128
        # partitions; classes side by side in columns: [128, NCLS*SW]
        w = out.reshape(NCLS, SW, 16).transpose(0, 2, 1)   # [NCLS, 16, SW]
        blocks = [np.tile(w[c], (8, 1)) for c in range(NCLS)]
        return np.concatenate(blocks, axis=1)

    return (wrapped(spair, sb), wrapped(dpair, db), part, col)


def kernel(z, edge_index, W1l, b1l, W1r, b1r, W2, b2):
    import ml_dtypes
    bf16 = ml_dtypes.bfloat16

    z = np.asarray(z, dtype=np.float32)
    edge_index = np.asarray(edge_index)
    W1l = np.asarray(W1l, dtype=np.float32)
    b1l = np.asarray(b1l, dtype=np.float32)
    W1r = np.asarray(W1r, dtype=np.float32)
    b1r = np.asarray(b1r, dtype=np.float32)
    W2 = np.asarray(W2, dtype=np.float32)
    b2 = np.asarray(b2, dtype=np.float32)

    # ---- host-side input prep ----
    zpad = np.zeros((NPAD, IN_CH), dtype=np.float32)
    zpad[:N_NODES] = z
    zT = np.ascontiguousarray(zpad.T).astype(bf16)           # [128, NPAD]

    w1lT = np.ascontiguousarray(W1l.T).astype(bf16)           # [128, 64]
    w1rT = np.ascontiguousarray(W1r.T).astype(bf16)
    ones_r = np.ones((1, P), dtype=bf16)
    bias_l = np.tile(b1l, 8)[None, :].astype(bf16)            # [1, 512]
    bias_r = np.tile(b1r, 8)[None, :].astype(bf16)
    w2full = np.tile(W2.reshape(1, HID), (P, C_SEG)).astype(np.float16)
    b2col = np.full((P, 1), float(b2[0]), dtype=np.float32)

    pi = _node_perm()
    src = pi[np.asarray(edge_index[0], dtype=np.int64)]
    dst = pi[np.asarray(edge_index[1], dtype=np.int64)]

    in_maps = []
    coords = []
    for c in range(N_CORES):
        sw, dw, part, col = _pack_core(src[c * E_CORE:(c + 1) * E_CORE],
                                       dst[c * E_CORE:(c + 1) * E_CORE])
        coords.append((part, col))
        in_maps.append({
            "zT": zT, "w1lT": w1lT, "w1rT": w1rT, "ones_r": ones_r,
            "bias_l": bias_l, "bias_r": bias_r, "w2full": w2full,
            "b2col": b2col, "src_w": sw, "dst_w": dw,
        })

    if "nc" not in _CACHE:
        _CACHE["nc"] = build_nc()
    nc = _CACHE["nc"]

    res = run_bass_kernel_spmd(nc, in_maps, list(range(N_CORES)))

    outs = []
    for c in range(N_CORES):
        part, col = coords[c]
        outs.append(res.results[c]["out"][part, col])
    return np.concatenate(outs).reshape(N_EDGES, 1).astype(np.float32)


if __name__ == "__main__":
    nc = build_nc()
    print("built OK")

